# revision 1
# baseline (speedup 1.0000x reference)
"""Multi-head attention (B=4, S=1024, DM=1024, H=16, D=64) on 8 Trainium2 cores.

Sharding: core c handles batch b = c//2 and heads [8*(c%2), 8*(c%2)+8)
(tensor-parallel over heads x data-parallel over batch).

fp32r trick: Trainium2's fp32r matmul dtype is fp32 with the mantissa
rounded to 11 bits (low 12 bits zero) and runs at full bf16 PE rate for
free dims >= 256. The BIR verifier requires fp32r matmul operands to be
*produced* as fp32r, so the host pre-rounds (RNE at bit 12) and
pre-transposes Q/K/V, and the DRAM inputs are declared float32r — DMA
lands matmul-ready tiles with no on-device transposes or conversion
copies.

Per core:
  1. DMA QT/KT/VT [DM, S] (host-pretransposed, fp32r) + W slices (fp32r).
  2. Projections (fp32r matmuls, N=512): QhT/KhT [128hd(head pair), S]
     = W^T X^T; Vh natural [k, hd] with an interleaved ones column per
     head ([Vh | 1] -> AV matmul also produces softmax denominators).
  3. Scores S^T[k, q] = KhT^T @ QhT per head, two heads row-packed in
     the PE array (K=64 at partition offsets 0/64) into one 2-bank PSUM
     tile. Causal: matmuls N-sliced to q >= k-block start; key-length
     mask is a per-partition bias in the Exp activation; P^T =
     Exp(S^T/8 + bias) written as fp32r, then the diagonal 128-col
     block is multiplied by a 0/1 triangle on gpsimd.
  4. O^T[d, q] (+ row 64 = softmax denominators) = [Vh | 1]^T @ P^T
     accumulated over k-blocks in PSUM. K projections are emitted per
     head pair so they overlap the ACT-bound softmax stream.
  5. recip = qmask / denominators; gpsimd partition-broadcast; DVE
     normalize; O^T DMA'd out (host transposes during reassembly).

No collectives: host shards inputs and reassembles the output.
"""

import sys

if "/opt/trn_rl_repo" not in sys.path:
    sys.path.insert(0, "/opt/trn_rl_repo")

from contextlib import ExitStack

import numpy as np

import concourse.bacc as bacc
import concourse.tile as tile
from concourse import mybir

B, S, DM, H, D = 4, 1024, 1024, 16, 64
NH = 512  # per-core output head-dims (8 heads x 64)
NEG = 1e12
f32, f32r = mybir.dt.float32, mybir.dt.float32r
Exp = mybir.ActivationFunctionType.Exp

_NC = None


def _emit(nc, tc, ctx, reps=1):
    Qd = nc.dram_tensor("QT", [128, 8, S], f32r, kind="ExternalInput")
    Kd = nc.dram_tensor("KT", [128, 8, S], f32r, kind="ExternalInput")
    Vd = nc.dram_tensor("VT", [128, 8, S], f32r, kind="ExternalInput")
    Wqd = nc.dram_tensor("Wq", [128, 8, NH], f32r, kind="ExternalInput")
    Wkd = nc.dram_tensor("Wk", [128, 8, NH], f32r, kind="ExternalInput")
    Wvd = nc.dram_tensor("Wv", [128, 8, NH], f32r, kind="ExternalInput")
    vbd = nc.dram_tensor("vbias", [128, 8], f32, kind="ExternalInput")
    qmd = nc.dram_tensor("qmask", [1, S], f32, kind="ExternalInput")
    Od = nc.dram_tensor("OT", [NH, S], f32, kind="ExternalOutput")

    cons = ctx.enter_context(tc.tile_pool(name="cons", bufs=1))
    xt_pool = ctx.enter_context(tc.tile_pool(name="xt", bufs=2))
    wpool = ctx.enter_context(tc.tile_pool(name="w", bufs=2))
    qk_pool = ctx.enter_context(tc.tile_pool(name="qk", bufs=2))
    vh_pool = ctx.enter_context(tc.tile_pool(name="vh", bufs=1))
    pt_pool = ctx.enter_context(tc.tile_pool(name="pt", bufs=9))
    nrm_pool = ctx.enter_context(tc.tile_pool(name="nrm", bufs=3))
    sm_pool = ctx.enter_context(tc.tile_pool(name="sm", bufs=4))
    ps_mm = ctx.enter_context(tc.tile_pool(name="psmm", bufs=2, space="PSUM"))
    ps_ot = ctx.enter_context(tc.tile_pool(name="psot", bufs=4, space="PSUM"))

    # tri01[p, t] = 1 if t >= p else 0  (zeroes q < k on the diagonal block,
    # applied multiplicatively to P = exp(S) on gpsimd after the exp)
    tri0 = cons.tile([128, 128], f32, tag="tri0")
    nc.vector.memset(tri0, 1.0)
    nc.gpsimd.affine_select(
        out=tri0,
        in_=tri0,
        compare_op=mybir.AluOpType.is_ge,
        fill=0.0,
        base=0,
        pattern=[[1, 128]],
        channel_multiplier=-1,
    )
    tri01 = cons.tile([128, 128], f32r, tag="tri01")
    nc.vector.tensor_copy(tri01, tri0)
    vb = cons.tile([128, 8], f32, tag="vb")
    nc.sync.dma_start(out=vb, in_=vbd.ap())
    qm = cons.tile([1, S], f32, tag="qm")
    nc.sync.dma_start(out=qm, in_=qmd.ap())
    ones0 = cons.tile([128, 1], f32, tag="ones0")
    nc.vector.memset(ones0, 1.0)
    onesf = cons.tile([128, 1], f32r, tag="onesf")
    nc.vector.tensor_copy(onesf, ones0)

    def body(rep):
        def load_xt(dram, eng2):
            # [DM, S] fp32r -> [128, 8 dm-chunks, S], 1MB per DMA, two rings
            xt = xt_pool.tile([128, 8, S], f32r, tag="xt", name="xt")
            view = dram.ap()
            for h in range(4):
                e = nc.sync if h % 2 == 0 else eng2
                e.dma_start(out=xt[:, 2 * h : 2 * h + 2], in_=view[:, 2 * h : 2 * h + 2])
            return xt

        def load_w(dram, eng, split=1):
            w = wpool.tile([128, 8, NH], f32r, tag="w", name="w")
            view = dram.ap()
            step = 8 // split
            for i in range(split):
                eng.dma_start(
                    out=w[:, i * step : (i + 1) * step],
                    in_=view[:, i * step : (i + 1) * step],
                )
            return w

        def project_qk(xt, w):
            out_t = qk_pool.tile([128, 4, S], f32r, tag="qk", name="qk")
            for hp in range(4):
                for qc in range(2):
                    pm = ps_mm.tile([128, 2, 512], f32, tag="mm", name="pm")[:, 0]
                    for c in range(8):
                        nc.tensor.matmul(
                            pm,
                            w[:, c, hp * 128 : (hp + 1) * 128],
                            xt[:, c, qc * 512 : (qc + 1) * 512],
                            start=(c == 0),
                            stop=(c == 7),
                        )
                    dst = out_t[:, hp, qc * 512 : (qc + 1) * 512]
                    if (hp + qc) % 2 == 0:
                        nc.vector.tensor_copy(out=dst, in_=pm)
                    else:
                        nc.scalar.copy(dst, pm)
            return out_t

        wq = load_w(Wqd, nc.sync, split=2)
        xtq = load_xt(Qd, nc.gpsimd)
        QhT = project_qk(xtq, wq)
        wk = load_w(Wkd, nc.gpsimd)
        xtk = load_xt(Kd, nc.gpsimd)
        KhT = project_qk(xtk, wk)

        def qk_exp_chunk(hp, qc, ki):
            off = max(0, ki * 128 - qc * 512)
            ksl = slice(ki * 128, (ki + 1) * 128)
            st = ps_mm.tile([128, 2, 512], f32, tag="mm", name="st")
            qk_off = min(off, 256)
            qk_qsl = slice(qc * 512 + qk_off, (qc + 1) * 512)
            for hx in range(2):
                hrow = slice(64 * hx, 64 * hx + 64)
                nc.tensor.matmul(
                    st[:, hx, qk_off:],
                    KhT[hrow, hp, ksl],
                    QhT[hrow, hp, qk_qsl],
                    start=True,
                    stop=True,
                )
            pt = pt_pool.tile([128, 2, 512], f32r, tag="pt", name="pt")
            nc.scalar.activation(
                pt[:, :, off:], st[:, :, off:], Exp,
                bias=vb[:, ki : ki + 1], scale=0.125,
            )
            if ki >= qc * 4:
                nc.gpsimd.tensor_mul(
                    pt[:, :, off : off + 128],
                    pt[:, :, off : off + 128],
                    tri01[:, None, :].to_broadcast([128, 2, 128]),
                )
            return pt

        # prologue: early score+exp chunks emitted before the V projection
        # so the ACT engine has softmax work during the PE-bound phase
        # (their AV consumers wait for VhO anyway)
        prologue = {
            (hp, 0, ki): qk_exp_chunk(hp, 0, ki)
            for hp in range(4)
            for ki in range(2)
        }

        # V last: AV(ki) needs only V-proj chunk kc=ki, so attention overlaps
        wv = load_w(Wvd, nc.gpsimd)
        xt = load_xt(Vd, nc.gpsimd)
        # V natural [k, hd] + ones column per head: [128k, 8 kc, 8 heads, 65]
        VhO = vh_pool.tile([128, 8, 8, 65], f32r, tag="vh", name="vh")
        nc.vector.tensor_copy(
            out=VhO[:, :, :, 64:65],
            in_=onesf[:, None, None, :].to_broadcast([128, 8, 8, 1]),
        )
        for kc in range(8):
            pm = ps_mm.tile([128, 2, 512], f32, tag="mm", name="pmv")[:, 0]
            for c in range(8):
                nc.tensor.matmul(
                    pm,
                    xt[:, c, kc * 128 : (kc + 1) * 128],
                    wv[:, c],
                    start=(c == 0),
                    stop=(c == 7),
                )
            nc.vector.tensor_copy(
                out=VhO[:, kc, :, 0:64],
                in_=pm.rearrange("p (h d) -> p h d", h=8),
            )

        # --- attention ---
        for hp in range(4):
            for qc in range(2):
                kmax = 4 * (qc + 1)
                ots = [
                    ps_ot.tile([128, 512], f32, tag="ot", name=f"ot{i}")[:65]
                    for i in range(2)
                ]
                for ki in range(kmax):
                    off = max(0, ki * 128 - qc * 512)
                    pt = prologue.pop((hp, qc, ki), None)
                    if pt is None:
                        pt = qk_exp_chunk(hp, qc, ki)
                    for hx in range(2):
                        nc.tensor.matmul(
                            ots[hx][:, off:],
                            VhO[:, ki, 2 * hp + hx, :],
                            pt[:, hx, off:],
                            start=(ki == 0),
                            stop=(ki == kmax - 1),
                        )
                for hx in range(2):
                    ot = ots[hx]
                    recip = sm_pool.tile([1, 512], f32, tag="sm", name="recip")
                    nc.vector.reciprocal(recip, ot[64:65, :])
                    nc.vector.tensor_mul(
                        recip, recip, qm[:, qc * 512 : (qc + 1) * 512]
                    )
                    rbc = nrm_pool.tile([64, 512], f32, tag="rbc", name="rbc")
                    nc.gpsimd.partition_broadcast(rbc, recip)
                    osb = nrm_pool.tile([64, 512], f32, tag="osb", name="osb")
                    nc.vector.tensor_mul(osb, ot[0:64, :], rbc)
                    hrow0 = hp * 128 + hx * 64
                    eng = nc.sync if hx == 0 else nc.scalar
                    eng.dma_start(
                        out=Od.ap()[hrow0 : hrow0 + 64, qc * 512 : (qc + 1) * 512],
                        in_=osb,
                    )

    if reps == 1:
        body(0)
    else:
        for r in range(reps):
            body(r)


def _build(reps=1):
    nc = bacc.Bacc("TRN2", target_bir_lowering=False, debug=False)
    with tile.TileContext(nc) as tc, ExitStack() as ctx:
        _emit(nc, tc, ctx, reps=reps)
    nc.compile()
    return nc


def get_nc(reps=1):
    global _NC
    if reps != 1:
        return _build(reps)
    if _NC is None:
        _NC = _build()
    return _NC


def _round_f32r(a):
    """Round fp32 to fp32r (11-bit mantissa, RNE) — what the PE consumes."""
    b = np.ascontiguousarray(a, np.float32).view(np.uint32)
    keep = b & np.uint32(0xFFFFF000)
    low = b & np.uint32(0xFFF)
    rnd = (low > 0x800) | ((low == 0x800) & (((b >> np.uint32(12)) & 1) == 1))
    out = keep + (rnd.astype(np.uint32) << np.uint32(12))
    return out.view(np.float32)


def make_in_maps(Q_seq, K_seq, V_seq, WQ, WK, WV, Q_len, V_len):
    karange = np.arange(S)
    in_maps = []
    def pack(xt_2d):
        # [DM, n] -> [128, 8, n], row c*128+p -> [p, c]
        return np.ascontiguousarray(
            xt_2d.reshape(8, 128, xt_2d.shape[1]).transpose(1, 0, 2)
        )

    qt = [_round_f32r(pack(Q_seq[b].T)) for b in range(B)]
    kt = [_round_f32r(pack(K_seq[b].T)) for b in range(B)]
    vt = [_round_f32r(pack(V_seq[b].T)) for b in range(B)]
    wq = [_round_f32r(pack(WQ[:, hh * NH : (hh + 1) * NH])) for hh in range(2)]
    wk = [_round_f32r(pack(WK[:, hh * NH : (hh + 1) * NH])) for hh in range(2)]
    wv = [_round_f32r(pack(WV[:, hh * NH : (hh + 1) * NH])) for hh in range(2)]
    for c in range(8):
        b, hh = c // 2, c % 2
        vbias = np.where(karange < int(V_len[b, 0]), 0.0, -NEG).astype(np.float32)
        qmask = (karange < int(Q_len[b, 0])).astype(np.float32)
        in_maps.append(
            {
                "QT": qt[b],
                "KT": kt[b],
                "VT": vt[b],
                "Wq": wq[hh],
                "Wk": wk[hh],
                "Wv": wv[hh],
                "vbias": np.ascontiguousarray(vbias.reshape(8, 128).T),
                "qmask": qmask[None, :],
            }
        )
    return in_maps


def assemble(results):
    out = np.empty((B, S, H * D), np.float32)
    for c in range(8):
        b, hh = c // 2, c % 2
        out[b, :, hh * NH : (hh + 1) * NH] = results[c]["OT"].T
    return out


def kernel(Q_seq, K_seq, V_seq, WQ, WK, WV, Q_len, V_len):
    from concourse.bass_utils import run_bass_kernel_spmd

    nc = get_nc()
    in_maps = make_in_maps(Q_seq, K_seq, V_seq, WQ, WK, WV, Q_len, V_len)
    r = run_bass_kernel_spmd(nc, in_maps, core_ids=list(range(8)))
    return assemble(r.results)



# revision 27
# speedup vs baseline: 2.0916x; 2.0916x over previous
"""Multi-head attention (B=4, S=1024, DM=1024, H=16, D=64) on 8 Trainium2 cores.

Sharding: core c handles batch b = c//2 and heads [8*(c%2), 8*(c%2)+8)
(tensor-parallel over heads x data-parallel over batch).

Length specialization: Q_len/V_len are runtime inputs, so the program is
compiled (and cached) per (QC, KC) = (ceil(max Q_len / 128),
ceil(max_b min(Q_len_b, V_len_b) / 128)) chunk counts. Work beyond those
bounds is exactly masked in the reference (causal + key mask + query mask),
so it is skipped: q >= 128*QC rows are zero (host fills), keys >= 128*KC
never attend. Per-batch residual masking stays data-driven: the host zeroes
V rows >= V_len and supplies a 0/1 "kones" column per key (so masked keys
add 0 to both the AV numerator and the softmax denominator), and the host
applies the query mask + softmax division during reassembly.

All PE operands are bf16 (full-rate at any free dim on TRN2; rel-err budget
2e-2 dwarfs bf16 rounding). Flow per core:
  1. DMA KT/QT/VT [128, 8dm, cols] and W slices (host pre-packed bf16).
  2. Per head pair hp: K-proj, Q-proj (PSUM accum over 8 dm chunks, copy to
     bf16 SBUF), then scores S^T[k,q] = KhT^T @ QhT per head (two heads
     row-packed at partition offsets 0/64) -> one PSUM tile; P^T =
     Exp(S^T/8) on ACT (bf16 out); diagonal 128-col blocks multiplied by a
     0/1 triangle on gpsimd. Interleaving keeps the ACT exp stream fed from
     ~7us while PE stays busy on projections.
  3. V-proj natural [k, hd] with the kones column appended per head
     ([Vh | kones] -> AV matmul also produces softmax denominators).
  4. O^T accumulated over k chunks in PSUM; [65, q] block (64 head dims +
     denominator row) copied to SBUF (DVE/Pool split) and DMA'd out
     unnormalized; the host divides by the denominator row, applies the
     query mask, and transposes during reassembly.

No collectives: host shards inputs and reassembles the output.
"""

import sys

if "/opt/trn_rl_repo" not in sys.path:
    sys.path.insert(0, "/opt/trn_rl_repo")

from contextlib import ExitStack

import numpy as np

import concourse.bacc as bacc
import concourse.tile as tile
from concourse import mybir

B, S, DM, H, D = 4, 1024, 1024, 16, 64
NH = 512  # per-core output head-dims (8 heads x 64)
f32, bf16 = mybir.dt.float32, mybir.dt.bfloat16
BF16_NP = mybir.dt.np(bf16)
Exp = mybir.ActivationFunctionType.Exp

_NC_CACHE = {}


def plan_from_lengths(Q_len, V_len):
    ql = max(int(q) for q in np.ravel(Q_len))
    kl = max(min(int(q), int(v)) for q, v in zip(np.ravel(Q_len), np.ravel(V_len)))
    QC = max(1, -(-ql // 128))
    KC = max(1, -(-kl // 128))
    return QC, KC


def _emit(nc, tc, ctx, QC, KC, reps=1):
    Qcols, Kcols = 128 * QC, 128 * KC
    qslices = [(s, min(512, Qcols - s)) for s in range(0, Qcols, 512)]
    kslices = [(s, min(512, Kcols - s)) for s in range(0, Kcols, 512)]

    Qd = nc.dram_tensor("QT", [128, 8, Qcols], bf16, kind="ExternalInput")
    Kd = nc.dram_tensor("KT", [128, 8, Kcols], bf16, kind="ExternalInput")
    Vd = nc.dram_tensor("VT", [128, 8, Kcols], bf16, kind="ExternalInput")
    # Wq/Wk are packed per head pair ([128, 4hp, 8c, 128]) so the first
    # projection can start after a 0.25MB DMA; Wv keeps the flat layout
    # (its V-proj matmul streams all 512 head-dims of one dm chunk)
    Wqd = nc.dram_tensor("Wq", [128, 4, 8, 128], bf16, kind="ExternalInput")
    Wkd = nc.dram_tensor("Wk", [128, 4, 8, 128], bf16, kind="ExternalInput")
    Wvd = nc.dram_tensor("Wv", [128, 8, NH], bf16, kind="ExternalInput")
    kod = nc.dram_tensor("kones", [128, KC], bf16, kind="ExternalInput")
    # per head hh=2*hp+hx: rows [65*hh, 65*hh+64) = O^T, row 65*hh+64 = denom
    Od = nc.dram_tensor("OT", [65 * 8, Qcols], f32, kind="ExternalOutput")

    cons = ctx.enter_context(tc.tile_pool(name="cons", bufs=1))
    xt_pool = ctx.enter_context(tc.tile_pool(name="xt", bufs=1))
    wpool = ctx.enter_context(tc.tile_pool(name="w", bufs=1))
    qk_pool = ctx.enter_context(tc.tile_pool(name="qk", bufs=1))
    vh_pool = ctx.enter_context(tc.tile_pool(name="vh", bufs=1))
    n_pt = 4 * sum(min(KC, (qs + qn) // 128) for qs, qn in qslices)
    pt_pool = ctx.enter_context(tc.tile_pool(name="pt", bufs=n_pt))
    osb_pool = ctx.enter_context(tc.tile_pool(name="osb", bufs=3))
    ps_pr = ctx.enter_context(tc.tile_pool(name="pspr", bufs=2, space="PSUM"))
    ps_sc = ctx.enter_context(tc.tile_pool(name="pssc", bufs=2, space="PSUM"))
    ps_av = ctx.enter_context(tc.tile_pool(name="psav", bufs=1, space="PSUM"))

    # tri01[p, t] = 1 if t >= p else 0  (zeroes q < k on the diagonal block,
    # applied multiplicatively to P = exp(S) on gpsimd after the exp)
    tri0 = cons.tile([128, 128], f32, tag="tri0")
    nc.vector.memset(tri0, 1.0)
    nc.gpsimd.affine_select(
        out=tri0,
        in_=tri0,
        compare_op=mybir.AluOpType.is_ge,
        fill=0.0,
        base=0,
        pattern=[[1, 128]],
        channel_multiplier=-1,
    )
    tri01 = cons.tile([128, 128], bf16, tag="tri01")
    nc.vector.tensor_copy(tri01, tri0)
    ko = cons.tile([128, KC], bf16, tag="ko")

    def body(rep):
        def load_x(dram, cols, tag, engs):
            # [DM, cols] bf16 -> [128, 8 dm-chunks, cols] in 4 DMAs
            xt = xt_pool.tile([128, 8, cols], bf16, tag=tag, name=tag)
            view = dram.ap()
            for h in range(4):
                engs[h % len(engs)].dma_start(
                    out=xt[:, 2 * h : 2 * h + 2], in_=view[:, 2 * h : 2 * h + 2]
                )
            return xt

        # Queue plan: sync/scalar are HWDGE queues sharing one ~400GB/s
        # engine; gpsimd is SWDGE (own bandwidth, ~1us/DMA descriptor-gen on
        # Pool). xtk (the Kp0 gate) and xtq split across all three; W per
        # head pair on sync; xtv (needed latest) on gpsimd.
        wk = wpool.tile([128, 4, 8, 128], bf16, tag="wk", name="wk")
        wq = wpool.tile([128, 4, 8, 128], bf16, tag="wq", name="wq")
        xtk = xt_pool.tile([128, 8, Kcols], bf16, tag="xtk", name="xtk")
        xtq = xt_pool.tile([128, 8, Qcols], bf16, tag="xtq", name="xtq")
        nc.sync.dma_start(out=wk[:, 0], in_=Wkd.ap()[:, 0])
        for h in range(4):
            eng = nc.scalar if h < 2 else nc.gpsimd
            eng.dma_start(
                out=xtk[:, 2 * h : 2 * h + 2], in_=Kd.ap()[:, 2 * h : 2 * h + 2]
            )
        nc.sync.dma_start(out=wq[:, 0], in_=Wqd.ap()[:, 0])
        for h in range(4):
            eng = nc.scalar if h < 2 else nc.gpsimd
            eng.dma_start(
                out=xtq[:, 2 * h : 2 * h + 2], in_=Qd.ap()[:, 2 * h : 2 * h + 2]
            )
        for hp in range(1, 4):
            nc.sync.dma_start(out=wk[:, hp], in_=Wkd.ap()[:, hp])
            nc.sync.dma_start(out=wq[:, hp], in_=Wqd.ap()[:, hp])
        wv = wpool.tile([128, 8, NH], bf16, tag="wv", name="wv")
        for i in range(2):
            nc.sync.dma_start(out=wv[:, 4 * i : 4 * i + 4], in_=Wvd.ap()[:, 4 * i : 4 * i + 4])
        if rep == 0:
            nc.sync.dma_start(out=ko, in_=kod.ap())
        xtv = xt_pool.tile([128, 8, Kcols], bf16, tag="xtv", name="xtv")
        for i in range(2):
            nc.scalar.dma_start(
                out=xtv[:, 4 * i : 4 * i + 4], in_=Vd.ap()[:, 4 * i : 4 * i + 4]
            )

        QhT = qk_pool.tile([128, 4, Qcols], bf16, tag="qh", name="QhT")
        KhT = qk_pool.tile([128, 4, Kcols], bf16, tag="kh", name="KhT")
        VhO = vh_pool.tile([128, KC, 8, 65], bf16, tag="vh", name="vh")

        def proj_slice(out_t, hp, xt, w, s_, n_):
            # out_t[:, hp, s:s+n] = (W^T X^T) for the 128 head-dims of hp
            pm = ps_pr.tile([128, 512], f32, tag="pr", name="pm")
            for c in range(8):
                nc.tensor.matmul(
                    pm[:, :n_],
                    w[:, hp, c],
                    xt[:, c, s_ : s_ + n_],
                    start=(c == 0),
                    stop=(c == 7),
                )
            nc.vector.tensor_copy(out=out_t[:, hp, s_ : s_ + n_], in_=pm[:, :n_])

        def vproj_chunk(kc):
            # V natural [k, hd]: VhO[:, kc, h, 0:64]
            pm = ps_pr.tile([128, 512], f32, tag="pr", name="pmv")
            for c in range(8):
                nc.tensor.matmul(
                    pm,
                    xtv[:, c, kc * 128 : (kc + 1) * 128],
                    wv[:, c],
                    start=(c == 0),
                    stop=(c == 7),
                )
            nc.vector.tensor_copy(
                out=VhO[:, kc, :, 0:64],
                in_=pm.rearrange("p (h d) -> p h d", h=8),
            )

        PT = {}

        def score_chunk(hp, qs, qn, ki):
            off = max(0, ki * 128 - qs)
            ksl = slice(ki * 128, (ki + 1) * 128)
            st = ps_sc.tile([128, 2, 512], f32, tag="sc", name="st")
            for hx in range(2):
                hrow = slice(64 * hx, 64 * hx + 64)
                nc.tensor.matmul(
                    st[:, hx, off:qn],
                    KhT[hrow, hp, ksl],
                    QhT[hrow, hp, qs + off : qs + qn],
                    start=True,
                    stop=True,
                )
            pt = pt_pool.tile([128, 2, 512], bf16, tag="pt", name="pt")
            nc.scalar.activation(pt[:, :, off:qn], st[:, :, off:qn], Exp, scale=0.125)
            if ki * 128 >= qs:
                # DVE, not Pool: Pool's in-order queue carries the osb copies,
                # which must not delay later tri muls (exp -> tri -> AV path)
                nc.vector.tensor_mul(
                    pt[:, :, off : off + 128],
                    pt[:, :, off : off + 128],
                    tri01[:, None, :].to_broadcast([128, 2, 128]),
                )
            PT[hp, qs, ki] = pt

        def av_group(hp, qs, qn, from_sc, act_helps):
            # O^T (+ denominator row 64) = [Vh | kones]^T @ P^T
            # after the score stream drains, its PSUM pool is free: tail
            # groups rotate through it instead of the single "av" buffer
            kmax = min(KC, (qs + qn) // 128)
            pool = ps_sc if from_sc else ps_av
            tag = "sc" if from_sc else "av"
            ot = pool.tile([128, 2, 512], f32, tag=tag, name="ot")
            for ki in range(kmax):
                off = max(0, ki * 128 - qs)
                pt = PT.pop((hp, qs, ki))
                for hx in range(2):
                    nc.tensor.matmul(
                        ot[:65, hx, off:qn],
                        VhO[:, ki, 2 * hp + hx, :],
                        pt[:, hx, off:qn],
                        start=(ki == 0),
                        stop=(ki == kmax - 1),
                    )
            osb = osb_pool.tile([65, 2, 512], f32, tag="osb", name="osb")
            if act_helps:
                # exp stream is drained by now: split the PSUM->SBUF copy
                # across DVE and ACT to halve the accumulator hold time
                nc.vector.tensor_copy(out=osb[:, 0, :qn], in_=ot[:65, 0, :qn])
                nc.scalar.copy(out=osb[:, 1, :qn], in_=ot[:65, 1, :qn])
            else:
                nc.vector.tensor_copy(out=osb[:, :, :qn], in_=ot[:65, :, :qn])
            for hx in range(2):
                hh = 2 * hp + hx
                nc.sync.dma_start(
                    out=Od.ap()[65 * hh : 65 * hh + 65, qs : qs + qn],
                    in_=osb[:, hx, :qn],
                )

        nc.vector.tensor_copy(
            out=VhO[:, :, :, 64:65],
            in_=ko[:, :, None, None].to_broadcast([128, KC, 8, 1]),
        )

        # --- interleaved emission ---------------------------------------
        # PE executes in order, so score chunks (whose PSUM slots gate on the
        # slower ACT exp stream) are spread between projection chunks and AV
        # groups; AV(g) is emitted once group g+1's scores are all emitted
        # (its exps are then strictly older than the score-slot frontier).
        def kq_items(hp):
            # K slice a, Q slice a, K slice b, Q slice b: the low-q score
            # group of hp unblocks after just the two `a` slices
            pairs = list(zip(kslices, qslices))
            items = []
            for i, (ks_, qs_) in enumerate(pairs):
                items += [("K", hp) + ks_, ("Q", hp) + qs_]
            items += [("K", hp) + s_ for s_ in kslices[len(pairs) :]]
            items += [("Q", hp) + s_ for s_ in qslices[len(pairs) :]]
            return items

        proj_rest = (
            kq_items(0)[2:]
            + kq_items(1)
            + [("V", kc) for kc in range(KC)]
            + kq_items(2)
            + kq_items(3)
        )
        groups = [(hp, qs, qn) for hp in range(4) for qs, qn in qslices]
        score_fifo = [
            (hp, qs, qn, ki)
            for hp, qs, qn in groups
            for ki in range(min(KC, (qs + qn) // 128))
        ]
        kdone = [0, 0, 0, 0]  # K cols projected per hp
        qdone = [set() for _ in range(4)]  # Q slice starts projected per hp
        v_done = [0]  # V chunks projected

        def emit_proj(item):
            if item[0] == "K":
                _, hp, s_, n_ = item
                proj_slice(KhT, hp, xtk, wk, s_, n_)
                kdone[hp] = s_ + n_
            elif item[0] == "Q":
                _, hp, s_, n_ = item
                proj_slice(QhT, hp, xtq, wq, s_, n_)
                qdone[hp].add(s_)
            else:
                vproj_chunk(item[1])
                v_done[0] += 1

        emit_proj(kq_items(0)[0])
        emit_proj(kq_items(0)[1])

        si = 0  # scores emitted
        pi = 0  # proj items emitted
        gi = 0  # AV groups emitted
        group_last_si = {}  # group index -> si after its last score chunk
        n_sc = len(score_fifo)

        def chunk_ready(hp, qs, qn, ki):
            return kdone[hp] >= (ki + 1) * 128 and qs in qdone[hp]

        while si < n_sc or gi < len(groups):
            progress = False
            # up to two score chunks whose projection slices are emitted
            for _ in range(2):
                if si < n_sc and chunk_ready(*score_fifo[si]):
                    hp, qs, qn, ki = score_fifo[si]
                    score_chunk(hp, qs, qn, ki)
                    si += 1
                    if ki == min(KC, (qs + qn) // 128) - 1:
                        group_last_si[len(group_last_si)] = si
                    progress = True
            # one projection item
            if pi < len(proj_rest):
                emit_proj(proj_rest[pi])
                pi += 1
                progress = True
            # AV group g once group g+1's scores are done (or none left) and
            # its V chunks are projected
            if gi < len(groups):
                hp, qs, qn = groups[gi]
                kmax = min(KC, (qs + qn) // 128)
                scores_ok = (gi + 1 in group_last_si) or (
                    si >= n_sc and gi in group_last_si
                )
                if scores_ok and v_done[0] >= kmax:
                    av_group(
                        hp,
                        qs,
                        qn,
                        from_sc=(si >= n_sc),
                        act_helps=(si >= n_sc and gi == len(groups) - 1),
                    )
                    gi += 1
                    progress = True
            assert progress, "interleave deadlock"

    for r in range(reps):
        body(r)


def _build(QC, KC, reps=1):
    nc = bacc.Bacc("TRN2", target_bir_lowering=False, debug=False)
    with tile.TileContext(nc) as tc, ExitStack() as ctx:
        _emit(nc, tc, ctx, QC, KC, reps=reps)
    nc.compile()
    return nc


def get_nc(reps=1, plan=(8, 8)):
    key = (plan, reps)
    if key not in _NC_CACHE:
        _NC_CACHE[key] = _build(*plan, reps=reps)
    return _NC_CACHE[key]


def make_in_maps(Q_seq, K_seq, V_seq, WQ, WK, WV, Q_len, V_len):
    QC, KC = plan_from_lengths(Q_len, V_len)
    Qcols, Kcols = 128 * QC, 128 * KC
    karange = np.arange(Kcols)

    def pack(xt_2d):
        # [DM, n] -> [128, 8, n], row c*128+p -> [p, c]
        return np.ascontiguousarray(
            xt_2d.reshape(8, 128, xt_2d.shape[1]).transpose(1, 0, 2).astype(BF16_NP)
        )

    def pack_hp(w_2d):
        # [128, 8c, 512] -> [128, 4hp, 8c, 128] (contiguous per-head-pair DMA)
        return np.ascontiguousarray(
            pack(w_2d).reshape(128, 8, 4, 128).transpose(0, 2, 1, 3)
        )

    qt = [pack(Q_seq[b].T[:, :Qcols]) for b in range(B)]
    kt = [pack(K_seq[b].T[:, :Kcols]) for b in range(B)]
    vt = []
    for b in range(B):
        v = np.asarray(V_seq[b]).copy()
        v[int(V_len[b, 0]) :] = 0.0  # masked keys contribute exactly 0
        vt.append(pack(v.T[:, :Kcols]))
    wq = [pack_hp(WQ[:, hh * NH : (hh + 1) * NH]) for hh in range(2)]
    wk = [pack_hp(WK[:, hh * NH : (hh + 1) * NH]) for hh in range(2)]
    wv = [pack(WV[:, hh * NH : (hh + 1) * NH]) for hh in range(2)]
    in_maps = []
    for c in range(8):
        b, hh = c // 2, c % 2
        kones = (karange < int(V_len[b, 0])).astype(np.float32)
        in_maps.append(
            {
                "QT": qt[b],
                "KT": kt[b],
                "VT": vt[b],
                "Wq": wq[hh],
                "Wk": wk[hh],
                "Wv": wv[hh],
                "kones": np.ascontiguousarray(
                    kones.reshape(KC, 128).T.astype(BF16_NP)
                ),
            }
        )
    return in_maps


def assemble(results, Q_len, plan):
    QC, _ = plan
    Qcols = 128 * QC
    out = np.zeros((B, S, H * D), np.float32)
    for c in range(8):
        b, hh = c // 2, c % 2
        r = results[c]["OT"].reshape(8, 65, Qcols)
        o = r[:, :64, :] / r[:, 64:65, :]  # [8, 64, Qcols] normalized
        ql = int(Q_len[b, 0])
        n = min(ql, Qcols)
        out[b, :n, hh * NH : (hh + 1) * NH] = (
            o[:, :, :n].transpose(2, 0, 1).reshape(n, NH)
        )
    return out


def kernel(Q_seq, K_seq, V_seq, WQ, WK, WV, Q_len, V_len):
    from concourse.bass_utils import run_bass_kernel_spmd

    plan = plan_from_lengths(Q_len, V_len)
    nc = get_nc(plan=plan)
    in_maps = make_in_maps(Q_seq, K_seq, V_seq, WQ, WK, WV, Q_len, V_len)
    r = run_bass_kernel_spmd(nc, in_maps, core_ids=list(range(8)))
    return assemble(r.results, Q_len, plan)


# revision 32
# speedup vs baseline: 2.1128x; 1.0101x over previous
"""Multi-head attention (B=4, S=1024, DM=1024, H=16, D=64) on 8 Trainium2 cores.

Sharding: core c handles batch b = c//2 and heads [8*(c%2), 8*(c%2)+8)
(tensor-parallel over heads x data-parallel over batch).

Length specialization: Q_len/V_len are runtime inputs, so the program is
compiled (and cached) per (QC, KC) = (ceil(max Q_len / 128),
ceil(max_b min(Q_len_b, V_len_b) / 128)) chunk counts. Work beyond those
bounds is exactly masked in the reference (causal + key mask + query mask),
so it is skipped: q >= 128*QC rows are zero (host fills), keys >= 128*KC
never attend. Per-batch residual masking stays data-driven: the host zeroes
V rows >= V_len and supplies a 0/1 "kones" column per key (so masked keys
add 0 to both the AV numerator and the softmax denominator), and the host
applies the query mask + softmax division during reassembly.

All PE operands are bf16 (full-rate at any free dim on TRN2; rel-err budget
2e-2 dwarfs bf16 rounding). Flow per core:
  1. DMA KT/QT/VT [128, 8dm, cols] and W slices (host pre-packed bf16).
  2. Per head pair hp: K-proj, Q-proj (PSUM accum over 8 dm chunks, copy to
     bf16 SBUF), then scores S^T[k,q] = KhT^T @ QhT per head (two heads
     row-packed at partition offsets 0/64) -> one PSUM tile; P^T =
     Exp(S^T/8) on ACT (bf16 out); diagonal 128-col blocks multiplied by a
     0/1 triangle on gpsimd. Interleaving keeps the ACT exp stream fed from
     ~7us while PE stays busy on projections.
  3. V-proj natural [k, hd] with the kones column appended per head
     ([Vh | kones] -> AV matmul also produces softmax denominators).
  4. O^T accumulated over k chunks in PSUM; [65, q] block (64 head dims +
     denominator row) copied to SBUF (DVE/Pool split) and DMA'd out
     unnormalized; the host divides by the denominator row, applies the
     query mask, and transposes during reassembly.

No collectives: host shards inputs and reassembles the output.
"""

import sys

if "/opt/trn_rl_repo" not in sys.path:
    sys.path.insert(0, "/opt/trn_rl_repo")

from contextlib import ExitStack

import numpy as np

import concourse.bacc as bacc
import concourse.tile as tile
from concourse import mybir

B, S, DM, H, D = 4, 1024, 1024, 16, 64
NH = 512  # per-core output head-dims (8 heads x 64)
f32, bf16 = mybir.dt.float32, mybir.dt.bfloat16
BF16_NP = mybir.dt.np(bf16)
Exp = mybir.ActivationFunctionType.Exp

_NC_CACHE = {}


def plan_from_lengths(Q_len, V_len):
    ql = max(int(q) for q in np.ravel(Q_len))
    kl = max(min(int(q), int(v)) for q, v in zip(np.ravel(Q_len), np.ravel(V_len)))
    QC = max(1, -(-ql // 128))
    KC = max(1, -(-kl // 128))
    return QC, KC


def _emit(nc, tc, ctx, QC, KC, reps=1):
    Qcols, Kcols = 128 * QC, 128 * KC
    qslices = [(s, min(512, Qcols - s)) for s in range(0, Qcols, 512)]
    kslices = [(s, min(512, Kcols - s)) for s in range(0, Kcols, 512)]

    Qd = nc.dram_tensor("QT", [128, 8, Qcols], bf16, kind="ExternalInput")
    Kd = nc.dram_tensor("KT", [128, 8, Kcols], bf16, kind="ExternalInput")
    Vd = nc.dram_tensor("VT", [128, 8, Kcols], bf16, kind="ExternalInput")
    # Wq/Wk are packed per head pair ([128, 4hp, 8c, 128]) so the first
    # projection can start after a 0.25MB DMA; Wv keeps the flat layout
    # (its V-proj matmul streams all 512 head-dims of one dm chunk)
    Wqd = nc.dram_tensor("Wq", [128, 4, 8, 128], bf16, kind="ExternalInput")
    Wkd = nc.dram_tensor("Wk", [128, 4, 8, 128], bf16, kind="ExternalInput")
    Wvd = nc.dram_tensor("Wv", [128, 8, NH], bf16, kind="ExternalInput")
    kod = nc.dram_tensor("kones", [128, KC], bf16, kind="ExternalInput")
    # head-pair blocks of 130 rows; within a block, row 2*p+hx holds head
    # hx's O^T dim p (p=64 -> denominator), so one DMA covers both heads
    Od = nc.dram_tensor("OT", [65 * 8, Qcols], f32, kind="ExternalOutput")

    cons = ctx.enter_context(tc.tile_pool(name="cons", bufs=1))
    xt_pool = ctx.enter_context(tc.tile_pool(name="xt", bufs=1))
    wpool = ctx.enter_context(tc.tile_pool(name="w", bufs=1))
    qk_pool = ctx.enter_context(tc.tile_pool(name="qk", bufs=1))
    vh_pool = ctx.enter_context(tc.tile_pool(name="vh", bufs=1))
    n_pt = 4 * sum(min(KC, (qs + qn) // 128) for qs, qn in qslices)
    pt_pool = ctx.enter_context(tc.tile_pool(name="pt", bufs=n_pt))
    osb_pool = ctx.enter_context(tc.tile_pool(name="osb", bufs=3))
    ps_pr = ctx.enter_context(tc.tile_pool(name="pspr", bufs=2, space="PSUM"))
    ps_sc = ctx.enter_context(tc.tile_pool(name="pssc", bufs=2, space="PSUM"))
    ps_av = ctx.enter_context(tc.tile_pool(name="psav", bufs=1, space="PSUM"))

    # tri01[p, t] = 1 if t >= p else 0  (zeroes q < k on the diagonal block,
    # applied multiplicatively to P = exp(S) on gpsimd after the exp)
    tri0 = cons.tile([128, 128], f32, tag="tri0")
    nc.vector.memset(tri0, 1.0)
    nc.gpsimd.affine_select(
        out=tri0,
        in_=tri0,
        compare_op=mybir.AluOpType.is_ge,
        fill=0.0,
        base=0,
        pattern=[[1, 128]],
        channel_multiplier=-1,
    )
    tri01 = cons.tile([128, 128], bf16, tag="tri01")
    nc.vector.tensor_copy(tri01, tri0)
    ko = cons.tile([128, KC], bf16, tag="ko")

    def body(rep):
        def load_x(dram, cols, tag, engs):
            # [DM, cols] bf16 -> [128, 8 dm-chunks, cols] in 4 DMAs
            xt = xt_pool.tile([128, 8, cols], bf16, tag=tag, name=tag)
            view = dram.ap()
            for h in range(4):
                engs[h % len(engs)].dma_start(
                    out=xt[:, 2 * h : 2 * h + 2], in_=view[:, 2 * h : 2 * h + 2]
                )
            return xt

        # Queue plan: sync/scalar are HWDGE queues sharing one ~400GB/s
        # engine; gpsimd is SWDGE (own bandwidth, ~1us/DMA descriptor-gen on
        # Pool). xtk (the Kp0 gate) and xtq split across all three; W per
        # head pair on sync; xtv (needed latest) on gpsimd.
        wk = wpool.tile([128, 4, 8, 128], bf16, tag="wk", name="wk")
        wq = wpool.tile([128, 4, 8, 128], bf16, tag="wq", name="wq")
        xtk = xt_pool.tile([128, 8, Kcols], bf16, tag="xtk", name="xtk")
        xtq = xt_pool.tile([128, 8, Qcols], bf16, tag="xtq", name="xtq")
        nc.sync.dma_start(out=wk[:, 0], in_=Wkd.ap()[:, 0])
        for h in range(4):
            eng = nc.scalar if h < 2 else nc.gpsimd
            eng.dma_start(
                out=xtk[:, 2 * h : 2 * h + 2], in_=Kd.ap()[:, 2 * h : 2 * h + 2]
            )
        nc.sync.dma_start(out=wq[:, 0], in_=Wqd.ap()[:, 0])
        for h in range(4):
            eng = nc.scalar if h < 2 else nc.gpsimd
            eng.dma_start(
                out=xtq[:, 2 * h : 2 * h + 2], in_=Qd.ap()[:, 2 * h : 2 * h + 2]
            )
        for hp in range(1, 4):
            nc.sync.dma_start(out=wk[:, hp], in_=Wkd.ap()[:, hp])
            nc.sync.dma_start(out=wq[:, hp], in_=Wqd.ap()[:, hp])
        wv = wpool.tile([128, 8, NH], bf16, tag="wv", name="wv")
        for i in range(2):
            nc.sync.dma_start(out=wv[:, 4 * i : 4 * i + 4], in_=Wvd.ap()[:, 4 * i : 4 * i + 4])
        if rep == 0:
            nc.sync.dma_start(out=ko, in_=kod.ap())
        xtv = xt_pool.tile([128, 8, Kcols], bf16, tag="xtv", name="xtv")
        for i in range(2):
            nc.scalar.dma_start(
                out=xtv[:, 4 * i : 4 * i + 4], in_=Vd.ap()[:, 4 * i : 4 * i + 4]
            )

        QhT = qk_pool.tile([128, 4, Qcols], bf16, tag="qh", name="QhT")
        KhT = qk_pool.tile([128, 4, Kcols], bf16, tag="kh", name="KhT")
        VhO = vh_pool.tile([128, KC, 8, 65], bf16, tag="vh", name="vh")

        def proj_slice(out_t, hp, xt, w, s_, n_):
            # out_t[:, hp, s:s+n] = (W^T X^T) for the 128 head-dims of hp
            pm = ps_pr.tile([128, 512], f32, tag="pr", name="pm")
            for c in range(8):
                nc.tensor.matmul(
                    pm[:, :n_],
                    w[:, hp, c],
                    xt[:, c, s_ : s_ + n_],
                    start=(c == 0),
                    stop=(c == 7),
                )
            nc.vector.tensor_copy(out=out_t[:, hp, s_ : s_ + n_], in_=pm[:, :n_])

        def vproj_chunk(kc):
            # V natural [k, hd]: VhO[:, kc, h, 0:64]
            pm = ps_pr.tile([128, 512], f32, tag="pr", name="pmv")
            for c in range(8):
                nc.tensor.matmul(
                    pm,
                    xtv[:, c, kc * 128 : (kc + 1) * 128],
                    wv[:, c],
                    start=(c == 0),
                    stop=(c == 7),
                )
            nc.vector.tensor_copy(
                out=VhO[:, kc, :, 0:64],
                in_=pm.rearrange("p (h d) -> p h d", h=8),
            )

        PT = {}

        def score_chunk(hp, qs, qn, ki):
            off = max(0, ki * 128 - qs)
            ksl = slice(ki * 128, (ki + 1) * 128)
            st = ps_sc.tile([128, 2, 512], f32, tag="sc", name="st")
            for hx in range(2):
                hrow = slice(64 * hx, 64 * hx + 64)
                nc.tensor.matmul(
                    st[:, hx, off:qn],
                    KhT[hrow, hp, ksl],
                    QhT[hrow, hp, qs + off : qs + qn],
                    start=True,
                    stop=True,
                )
            pt = pt_pool.tile([128, 2, 512], bf16, tag="pt", name="pt")
            nc.scalar.activation(pt[:, :, off:qn], st[:, :, off:qn], Exp, scale=0.125)
            if ki * 128 >= qs:
                # DVE, not Pool: Pool's in-order queue carries the osb copies,
                # which must not delay later tri muls (exp -> tri -> AV path)
                nc.vector.tensor_mul(
                    pt[:, :, off : off + 128],
                    pt[:, :, off : off + 128],
                    tri01[:, None, :].to_broadcast([128, 2, 128]),
                )
            PT[hp, qs, ki] = pt

        def av_group(hp, qs, qn, from_sc, act_helps):
            # O^T (+ denominator row 64) = [Vh | kones]^T @ P^T
            # after the score stream drains, its PSUM pool is free: tail
            # groups rotate through it instead of the single "av" buffer
            kmax = min(KC, (qs + qn) // 128)
            pool = ps_sc if from_sc else ps_av
            tag = "sc" if from_sc else "av"
            ot = pool.tile([128, 2, 512], f32, tag=tag, name="ot")
            for ki in range(kmax):
                off = max(0, ki * 128 - qs)
                pt = PT.pop((hp, qs, ki))
                for hx in range(2):
                    nc.tensor.matmul(
                        ot[:65, hx, off:qn],
                        VhO[:, ki, 2 * hp + hx, :],
                        pt[:, hx, off:qn],
                        start=(ki == 0),
                        stop=(ki == kmax - 1),
                    )
            osb = osb_pool.tile([65, 2, 512], f32, tag="osb", name="osb")
            if act_helps:
                # exp stream is drained by now: split the PSUM->SBUF copy
                # across DVE and ACT to halve the accumulator hold time
                nc.vector.tensor_copy(out=osb[:, 0, :qn], in_=ot[:65, 0, :qn])
                nc.scalar.copy(out=osb[:, 1, :qn], in_=ot[:65, 1, :qn])
            else:
                nc.vector.tensor_copy(out=osb[:, :, :qn], in_=ot[:65, :, :qn])
            nc.sync.dma_start(
                out=Od.ap()[130 * hp : 130 * hp + 130, qs : qs + qn],
                in_=osb[:, :, :qn],
            )

        nc.vector.tensor_copy(
            out=VhO[:, :, :, 64:65],
            in_=ko[:, :, None, None].to_broadcast([128, KC, 8, 1]),
        )

        # --- interleaved emission ---------------------------------------
        # PE executes in order, so score chunks (whose PSUM slots gate on the
        # slower ACT exp stream) are spread between projection chunks and AV
        # groups; AV(g) is emitted once group g+1's scores are all emitted
        # (its exps are then strictly older than the score-slot frontier).
        def kq_items(hp):
            # K slice a, Q slice a, K slice b, Q slice b: the low-q score
            # group of hp unblocks after just the two `a` slices
            pairs = list(zip(kslices, qslices))
            items = []
            for i, (ks_, qs_) in enumerate(pairs):
                items += [("K", hp) + ks_, ("Q", hp) + qs_]
            items += [("K", hp) + s_ for s_ in kslices[len(pairs) :]]
            items += [("Q", hp) + s_ for s_ in qslices[len(pairs) :]]
            return items

        proj_rest = (
            kq_items(0)[2:]
            + kq_items(1)
            + [("V", kc) for kc in range(KC)]
            + kq_items(2)
            + kq_items(3)
        )
        groups = [(hp, qs, qn) for hp in range(4) for qs, qn in qslices]
        score_fifo = [
            (hp, qs, qn, ki)
            for hp, qs, qn in groups
            for ki in range(min(KC, (qs + qn) // 128))
        ]
        kdone = [0, 0, 0, 0]  # K cols projected per hp
        qdone = [set() for _ in range(4)]  # Q slice starts projected per hp
        v_done = [0]  # V chunks projected

        def emit_proj(item):
            if item[0] == "K":
                _, hp, s_, n_ = item
                proj_slice(KhT, hp, xtk, wk, s_, n_)
                kdone[hp] = s_ + n_
            elif item[0] == "Q":
                _, hp, s_, n_ = item
                proj_slice(QhT, hp, xtq, wq, s_, n_)
                qdone[hp].add(s_)
            else:
                vproj_chunk(item[1])
                v_done[0] += 1

        emit_proj(kq_items(0)[0])
        emit_proj(kq_items(0)[1])

        si = 0  # scores emitted
        pi = 0  # proj items emitted
        gi = 0  # AV groups emitted
        group_last_si = {}  # group index -> si after its last score chunk
        n_sc = len(score_fifo)

        def chunk_ready(hp, qs, qn, ki):
            return kdone[hp] >= (ki + 1) * 128 and qs in qdone[hp]

        while si < n_sc or gi < len(groups):
            progress = False
            # up to two score chunks whose projection slices are emitted
            for _ in range(2):
                if si < n_sc and chunk_ready(*score_fifo[si]):
                    hp, qs, qn, ki = score_fifo[si]
                    score_chunk(hp, qs, qn, ki)
                    si += 1
                    if ki == min(KC, (qs + qn) // 128) - 1:
                        group_last_si[len(group_last_si)] = si
                    progress = True
            # one projection item
            if pi < len(proj_rest):
                emit_proj(proj_rest[pi])
                pi += 1
                progress = True
            # AV group g once group g+1's scores are done (or none left) and
            # its V chunks are projected
            if gi < len(groups):
                hp, qs, qn = groups[gi]
                kmax = min(KC, (qs + qn) // 128)
                scores_ok = (gi + 1 in group_last_si) or (
                    si >= n_sc and gi in group_last_si
                )
                if scores_ok and v_done[0] >= kmax:
                    av_group(
                        hp,
                        qs,
                        qn,
                        from_sc=(si >= n_sc),
                        act_helps=(si >= n_sc and gi == len(groups) - 1),
                    )
                    gi += 1
                    progress = True
            assert progress, "interleave deadlock"

    for r in range(reps):
        body(r)


def _build(QC, KC, reps=1):
    nc = bacc.Bacc("TRN2", target_bir_lowering=False, debug=False)
    with tile.TileContext(nc) as tc, ExitStack() as ctx:
        _emit(nc, tc, ctx, QC, KC, reps=reps)
    nc.compile()
    return nc


def get_nc(reps=1, plan=(8, 8)):
    key = (plan, reps)
    if key not in _NC_CACHE:
        _NC_CACHE[key] = _build(*plan, reps=reps)
    return _NC_CACHE[key]


def make_in_maps(Q_seq, K_seq, V_seq, WQ, WK, WV, Q_len, V_len):
    QC, KC = plan_from_lengths(Q_len, V_len)
    Qcols, Kcols = 128 * QC, 128 * KC
    karange = np.arange(Kcols)

    def pack(xt_2d):
        # [DM, n] -> [128, 8, n], row c*128+p -> [p, c]
        return np.ascontiguousarray(
            xt_2d.reshape(8, 128, xt_2d.shape[1]).transpose(1, 0, 2).astype(BF16_NP)
        )

    def pack_hp(w_2d):
        # [128, 8c, 512] -> [128, 4hp, 8c, 128] (contiguous per-head-pair DMA)
        return np.ascontiguousarray(
            pack(w_2d).reshape(128, 8, 4, 128).transpose(0, 2, 1, 3)
        )

    qt = [pack(Q_seq[b].T[:, :Qcols]) for b in range(B)]
    kt = [pack(K_seq[b].T[:, :Kcols]) for b in range(B)]
    vt = []
    for b in range(B):
        v = np.asarray(V_seq[b]).copy()
        v[int(V_len[b, 0]) :] = 0.0  # masked keys contribute exactly 0
        vt.append(pack(v.T[:, :Kcols]))
    wq = [pack_hp(WQ[:, hh * NH : (hh + 1) * NH]) for hh in range(2)]
    wk = [pack_hp(WK[:, hh * NH : (hh + 1) * NH]) for hh in range(2)]
    wv = [pack(WV[:, hh * NH : (hh + 1) * NH]) for hh in range(2)]
    in_maps = []
    for c in range(8):
        b, hh = c // 2, c % 2
        kones = (karange < int(V_len[b, 0])).astype(np.float32)
        in_maps.append(
            {
                "QT": qt[b],
                "KT": kt[b],
                "VT": vt[b],
                "Wq": wq[hh],
                "Wk": wk[hh],
                "Wv": wv[hh],
                "kones": np.ascontiguousarray(
                    kones.reshape(KC, 128).T.astype(BF16_NP)
                ),
            }
        )
    return in_maps


def assemble(results, Q_len, plan):
    QC, _ = plan
    Qcols = 128 * QC
    out = np.zeros((B, S, H * D), np.float32)
    for c in range(8):
        b, hh = c // 2, c % 2
        # row layout: [4 hp, 65 p, 2 hx]; p=64 is the denominator row
        r = results[c]["OT"].reshape(4, 65, 2, Qcols)
        o = r[:, :64] / r[:, 64:65]  # [4, 64, 2, Qcols] normalized
        ql = int(Q_len[b, 0])
        n = min(ql, Qcols)
        # out col for head 2*hp+hx, dim d = (2*hp+hx)*64 + d
        out[b, :n, hh * NH : (hh + 1) * NH] = (
            o[:, :, :, :n].transpose(3, 0, 2, 1).reshape(n, NH)
        )
    return out


def kernel(Q_seq, K_seq, V_seq, WQ, WK, WV, Q_len, V_len):
    from concourse.bass_utils import run_bass_kernel_spmd

    plan = plan_from_lengths(Q_len, V_len)
    nc = get_nc(plan=plan)
    in_maps = make_in_maps(Q_seq, K_seq, V_seq, WQ, WK, WV, Q_len, V_len)
    r = run_bass_kernel_spmd(nc, in_maps, core_ids=list(range(8)))
    return assemble(r.results, Q_len, plan)


# revision 42
# speedup vs baseline: 2.3482x; 1.1114x over previous
"""Multi-head attention (B=4, S=1024, DM=1024, H=16, D=64) on 8 Trainium2 cores.

Sharding: core c handles batch b = c//2 and heads [8*(c%2), 8*(c%2)+8)
(tensor-parallel over heads x data-parallel over batch).

Length specialization: Q_len/V_len are runtime inputs, so the program is
compiled (and cached) per (QC, KC) = (ceil(max Q_len / 128),
ceil(max_b min(Q_len_b, V_len_b) / 128)) chunk counts. Work beyond those
bounds is exactly masked in the reference (causal + key mask + query mask),
so it is skipped: q >= 128*QC rows are zero (host fills), keys >= 128*KC
never attend. Per-batch residual masking stays data-driven: the host zeroes
V rows >= V_len and supplies a 0/1 "kones" column per key (so masked keys
add 0 to both the AV numerator and the softmax denominator), and the host
applies the query mask + softmax division during reassembly.

All PE operands are bf16 (full-rate at any free dim on TRN2; rel-err budget
2e-2 dwarfs bf16 rounding). Flow per core:
  1. DMA KT/QT/VT [128, 8dm, cols] and W slices (host pre-packed bf16).
  2. Per head pair hp: K-proj, Q-proj (PSUM accum over 8 dm chunks, copy to
     bf16 SBUF), then scores S^T[k,q] = KhT^T @ QhT per head (two heads
     row-packed at partition offsets 0/64) -> one PSUM tile; P^T =
     Exp(S^T/8) on ACT (bf16 out); diagonal 128-col blocks multiplied by a
     0/1 triangle on gpsimd. Interleaving keeps the ACT exp stream fed from
     ~7us while PE stays busy on projections.
  3. V-proj natural [k, hd] with the kones column appended per head
     ([Vh | kones] -> AV matmul also produces softmax denominators).
  4. O^T accumulated over k chunks in PSUM; the [65, 2, q] block (64 head
     dims + denominator row, both heads) is copied to SBUF and DMA'd out
     unnormalized in one transfer; the host divides by the denominator
     row, applies the query mask, and transposes during reassembly.

No collectives: host shards inputs and reassembles the output.
"""

import sys

if "/opt/trn_rl_repo" not in sys.path:
    sys.path.insert(0, "/opt/trn_rl_repo")

from contextlib import ExitStack

import numpy as np

import concourse.bacc as bacc
import concourse.tile as tile
from concourse import mybir

B, S, DM, H, D = 4, 1024, 1024, 16, 64
NH = 512  # per-core output head-dims (8 heads x 64)
f32, bf16 = mybir.dt.float32, mybir.dt.bfloat16
BF16_NP = mybir.dt.np(bf16)
Exp = mybir.ActivationFunctionType.Exp

_NC_CACHE = {}


def plan_from_lengths(Q_len, V_len):
    ql = max(int(q) for q in np.ravel(Q_len))
    kl = max(min(int(q), int(v)) for q, v in zip(np.ravel(Q_len), np.ravel(V_len)))
    QC = max(1, -(-ql // 128))
    KC = max(1, -(-kl // 128))
    return QC, KC


def _emit(nc, tc, ctx, QC, KC, reps=1):
    Qcols, Kcols = 128 * QC, 128 * KC
    qslices = [(s, min(512, Qcols - s)) for s in range(0, Qcols, 512)]
    kslices = [(s, min(512, Kcols - s)) for s in range(0, Kcols, 512)]

    Qd = nc.dram_tensor("QT", [128, 8, Qcols], bf16, kind="ExternalInput")
    Kd = nc.dram_tensor("KT", [128, 8, Kcols], bf16, kind="ExternalInput")
    Vd = nc.dram_tensor("VT", [128, 8, Kcols], bf16, kind="ExternalInput")
    # Wk/Wq are stacked per head pair ([128, 4hp, 2kq, 8c, 128]) so one
    # 0.5MB DMA delivers both projections of a head pair; Wv keeps the flat
    # layout (its V-proj matmul streams all 512 head-dims of one dm chunk)
    Wkqd = nc.dram_tensor("Wkq", [128, 4, 2, 8, 128], bf16, kind="ExternalInput")
    Wvd = nc.dram_tensor("Wv", [128, 8, NH], bf16, kind="ExternalInput")
    kod = nc.dram_tensor("kones", [128, KC], bf16, kind="ExternalInput")
    # head-pair blocks of 130 rows; within a block, row 2*p+hx holds head
    # hx's O^T dim p (p=64 -> denominator), so one DMA covers both heads
    Od = nc.dram_tensor("OT", [65 * 8, Qcols], f32, kind="ExternalOutput")

    cons = ctx.enter_context(tc.tile_pool(name="cons", bufs=1))
    xt_pool = ctx.enter_context(tc.tile_pool(name="xt", bufs=1))
    wpool = ctx.enter_context(tc.tile_pool(name="w", bufs=1))
    qk_pool = ctx.enter_context(tc.tile_pool(name="qk", bufs=1))
    vh_pool = ctx.enter_context(tc.tile_pool(name="vh", bufs=1))
    n_pt = 4 * sum(min(KC, (qs + qn) // 128) for qs, qn in qslices)
    pt_pool = ctx.enter_context(tc.tile_pool(name="pt", bufs=n_pt))
    osb_pool = ctx.enter_context(tc.tile_pool(name="osb", bufs=3))
    ps_pr = ctx.enter_context(tc.tile_pool(name="pspr", bufs=2, space="PSUM"))
    ps_sc = ctx.enter_context(tc.tile_pool(name="pssc", bufs=2, space="PSUM"))
    ps_av = ctx.enter_context(tc.tile_pool(name="psav", bufs=1, space="PSUM"))

    # tri01[p, t] = 1 if t >= p else 0  (zeroes q < k on the diagonal block,
    # applied multiplicatively to P = exp(S) on gpsimd after the exp)
    tri0 = cons.tile([128, 128], f32, tag="tri0")
    nc.vector.memset(tri0, 1.0)
    nc.gpsimd.affine_select(
        out=tri0,
        in_=tri0,
        compare_op=mybir.AluOpType.is_ge,
        fill=0.0,
        base=0,
        pattern=[[1, 128]],
        channel_multiplier=-1,
    )
    tri01 = cons.tile([128, 128], bf16, tag="tri01")
    nc.vector.tensor_copy(tri01, tri0)
    ko = cons.tile([128, KC], bf16, tag="ko")

    def body(rep):
        # Queue plan: sync/scalar are HWDGE queues sharing one ~400GB/s
        # engine; gpsimd is SWDGE (own bandwidth, ~1us/DMA descriptor-gen on
        # Pool). Dispatch cost (~0.7-1.3us/DMA) dominates small transfers,
        # so chunk counts scale with size. xtk/xtq split scalar/gpsimd; W
        # stream on sync; xtv on scalar after xtq.
        wkq = wpool.tile([128, 4, 2, 8, 128], bf16, tag="wkq", name="wkq")
        xtk = xt_pool.tile([128, 8, Kcols], bf16, tag="xtk", name="xtk")
        xtq = xt_pool.tile([128, 8, Qcols], bf16, tag="xtq", name="xtq")

        def load_split(xt, dram):
            for h in range(4):
                eng = nc.scalar if h < 2 else nc.gpsimd
                eng.dma_start(
                    out=xt[:, 2 * h : 2 * h + 2],
                    in_=dram.ap()[:, 2 * h : 2 * h + 2],
                )

        nc.sync.dma_start(out=wkq[:, 0, 0], in_=Wkqd.ap()[:, 0, 0])
        load_split(xtk, Kd)
        nc.sync.dma_start(out=wkq[:, 0, 1], in_=Wkqd.ap()[:, 0, 1])
        load_split(xtq, Qd)
        for hp in range(1, 4):
            for kq in range(2):
                nc.sync.dma_start(out=wkq[:, hp, kq], in_=Wkqd.ap()[:, hp, kq])
        wv = wpool.tile([128, 8, NH], bf16, tag="wv", name="wv")
        for i in range(2):
            nc.sync.dma_start(out=wv[:, 4 * i : 4 * i + 4], in_=Wvd.ap()[:, 4 * i : 4 * i + 4])
        if rep == 0:
            nc.sync.dma_start(out=ko, in_=kod.ap())
        xtv = xt_pool.tile([128, 8, Kcols], bf16, tag="xtv", name="xtv")
        for i in range(2):
            nc.scalar.dma_start(
                out=xtv[:, 4 * i : 4 * i + 4], in_=Vd.ap()[:, 4 * i : 4 * i + 4]
            )
        wk = wkq[:, :, 0]
        wq = wkq[:, :, 1]

        QhT = qk_pool.tile([128, 4, Qcols], bf16, tag="qh", name="QhT")
        KhT = qk_pool.tile([128, 4, Kcols], bf16, tag="kh", name="KhT")
        VhO = vh_pool.tile([128, KC, 8, 65], bf16, tag="vh", name="vh")

        def proj_slice(out_t, hp, xt, w, s_, n_):
            # out_t[:, hp, s:s+n] = (W^T X^T) for the 128 head-dims of hp
            pm = ps_pr.tile([128, 512], f32, tag="pr", name="pm")
            for c in range(8):
                nc.tensor.matmul(
                    pm[:, :n_],
                    w[:, hp, c],
                    xt[:, c, s_ : s_ + n_],
                    start=(c == 0),
                    stop=(c == 7),
                )
            nc.vector.tensor_copy(out=out_t[:, hp, s_ : s_ + n_], in_=pm[:, :n_])

        def vproj_chunk(kc):
            # V natural [k, hd]: VhO[:, kc, h, 0:64]
            pm = ps_pr.tile([128, 512], f32, tag="pr", name="pmv")
            for c in range(8):
                nc.tensor.matmul(
                    pm,
                    xtv[:, c, kc * 128 : (kc + 1) * 128],
                    wv[:, c],
                    start=(c == 0),
                    stop=(c == 7),
                )
            nc.vector.tensor_copy(
                out=VhO[:, kc, :, 0:64],
                in_=pm.rearrange("p (h d) -> p h d", h=8),
            )

        PT = {}

        def score_chunk(hp, qs, qn, ki):
            off = max(0, ki * 128 - qs)
            ksl = slice(ki * 128, (ki + 1) * 128)
            st = ps_sc.tile([128, 2, 512], f32, tag="sc", name="st")
            for hx in range(2):
                hrow = slice(64 * hx, 64 * hx + 64)
                nc.tensor.matmul(
                    st[:, hx, off:qn],
                    KhT[hrow, hp, ksl],
                    QhT[hrow, hp, qs + off : qs + qn],
                    start=True,
                    stop=True,
                )
            pt = pt_pool.tile([128, 2, 512], bf16, tag="pt", name="pt")
            nc.scalar.activation(pt[:, :, off:qn], st[:, :, off:qn], Exp, scale=0.125)
            if ki * 128 >= qs:
                # DVE, not Pool: Pool's in-order queue carries the osb copies,
                # which must not delay later tri muls (exp -> tri -> AV path)
                nc.vector.tensor_mul(
                    pt[:, :, off : off + 128],
                    pt[:, :, off : off + 128],
                    tri01[:, None, :].to_broadcast([128, 2, 128]),
                )
            PT[hp, qs, ki] = pt

        def av_group(hp, qs, qn, from_sc, act_helps):
            # O^T (+ denominator row 64) = [Vh | kones]^T @ P^T
            # after the score stream drains, its PSUM pool is free: tail
            # groups rotate through it instead of the single "av" buffer
            kmax = min(KC, (qs + qn) // 128)
            pool = ps_sc if from_sc else ps_av
            tag = "sc" if from_sc else "av"
            ot = pool.tile([128, 2, 512], f32, tag=tag, name="ot")
            for ki in range(kmax):
                off = max(0, ki * 128 - qs)
                pt = PT.pop((hp, qs, ki))
                for hx in range(2):
                    nc.tensor.matmul(
                        ot[:65, hx, off:qn],
                        VhO[:, ki, 2 * hp + hx, :],
                        pt[:, hx, off:qn],
                        start=(ki == 0),
                        stop=(ki == kmax - 1),
                    )
            osb = osb_pool.tile([65, 2, 512], f32, tag="osb", name="osb")
            if act_helps:
                # exp stream is drained by now: split the PSUM->SBUF copy
                # across DVE and ACT to halve the accumulator hold time
                nc.vector.tensor_copy(out=osb[:, 0, :qn], in_=ot[:65, 0, :qn])
                nc.scalar.copy(out=osb[:, 1, :qn], in_=ot[:65, 1, :qn])
            else:
                nc.vector.tensor_copy(out=osb[:, :, :qn], in_=ot[:65, :, :qn])
            nc.sync.dma_start(
                out=Od.ap()[130 * hp : 130 * hp + 130, qs : qs + qn],
                in_=osb[:, :, :qn],
            )

        nc.vector.tensor_copy(
            out=VhO[:, :, :, 64:65],
            in_=ko[:, :, None, None].to_broadcast([128, KC, 8, 1]),
        )

        # --- interleaved emission ---------------------------------------
        # PE executes in order, so score chunks (whose PSUM slots gate on the
        # slower ACT exp stream) are spread between projection chunks and AV
        # groups; AV(g) is emitted once group g+1's scores are all emitted
        # (its exps are then strictly older than the score-slot frontier).
        def kq_items(hp):
            # K slice a, Q slice a, K slice b, Q slice b: the low-q score
            # group of hp unblocks after just the two `a` slices
            pairs = list(zip(kslices, qslices))
            items = []
            for i, (ks_, qs_) in enumerate(pairs):
                items += [("K", hp) + ks_, ("Q", hp) + qs_]
            items += [("K", hp) + s_ for s_ in kslices[len(pairs) :]]
            items += [("Q", hp) + s_ for s_ in qslices[len(pairs) :]]
            return items

        proj_rest = (
            kq_items(0)[2:]
            + kq_items(1)
            + [("V", kc) for kc in range(KC)]
            + kq_items(2)
            + kq_items(3)
        )
        groups = [(hp, qs, qn) for hp in range(4) for qs, qn in qslices]
        score_fifo = [
            (hp, qs, qn, ki)
            for hp, qs, qn in groups
            for ki in range(min(KC, (qs + qn) // 128))
        ]
        kdone = [0, 0, 0, 0]  # K cols projected per hp
        qdone = [set() for _ in range(4)]  # Q slice starts projected per hp
        v_done = [0]  # V chunks projected

        def emit_proj(item):
            if item[0] == "K":
                _, hp, s_, n_ = item
                proj_slice(KhT, hp, xtk, wk, s_, n_)
                kdone[hp] = s_ + n_
            elif item[0] == "Q":
                _, hp, s_, n_ = item
                proj_slice(QhT, hp, xtq, wq, s_, n_)
                qdone[hp].add(s_)
            else:
                vproj_chunk(item[1])
                v_done[0] += 1

        emit_proj(kq_items(0)[0])
        emit_proj(kq_items(0)[1])

        si = 0  # scores emitted
        pi = 0  # proj items emitted
        gi = 0  # AV groups emitted
        group_last_si = {}  # group index -> si after its last score chunk
        n_sc = len(score_fifo)

        def chunk_ready(hp, qs, qn, ki):
            return kdone[hp] >= (ki + 1) * 128 and qs in qdone[hp]

        while si < n_sc or gi < len(groups):
            progress = False
            # up to two score chunks whose projection slices are emitted
            for _ in range(2):
                if si < n_sc and chunk_ready(*score_fifo[si]):
                    hp, qs, qn, ki = score_fifo[si]
                    score_chunk(hp, qs, qn, ki)
                    si += 1
                    if ki == min(KC, (qs + qn) // 128) - 1:
                        group_last_si[len(group_last_si)] = si
                    progress = True
            # one projection item
            if pi < len(proj_rest):
                emit_proj(proj_rest[pi])
                pi += 1
                progress = True
            # AV group g once group g+1's scores are done (or none left) and
            # its V chunks are projected
            if gi < len(groups):
                hp, qs, qn = groups[gi]
                kmax = min(KC, (qs + qn) // 128)
                scores_ok = (gi + 1 in group_last_si) or (
                    si >= n_sc and gi in group_last_si
                )
                if scores_ok and v_done[0] >= kmax:
                    av_group(
                        hp,
                        qs,
                        qn,
                        from_sc=(si >= n_sc and gi % 2 == 0),
                        act_helps=(si >= n_sc and gi == len(groups) - 1),
                    )
                    gi += 1
                    progress = True
            assert progress, "interleave deadlock"

    for r in range(reps):
        body(r)


def _build(QC, KC, reps=1):
    nc = bacc.Bacc("TRN2", target_bir_lowering=False, debug=False)
    with tile.TileContext(nc) as tc, ExitStack() as ctx:
        _emit(nc, tc, ctx, QC, KC, reps=reps)
    nc.compile()
    return nc


def get_nc(reps=1, plan=(8, 8)):
    key = (plan, reps)
    if key not in _NC_CACHE:
        _NC_CACHE[key] = _build(*plan, reps=reps)
    return _NC_CACHE[key]


def make_in_maps(Q_seq, K_seq, V_seq, WQ, WK, WV, Q_len, V_len):
    QC, KC = plan_from_lengths(Q_len, V_len)
    Qcols, Kcols = 128 * QC, 128 * KC
    karange = np.arange(Kcols)

    def pack(xt_2d):
        # [DM, n] -> [128, 8, n], row c*128+p -> [p, c]
        return np.ascontiguousarray(
            xt_2d.reshape(8, 128, xt_2d.shape[1]).transpose(1, 0, 2).astype(BF16_NP)
        )

    def pack_hp(w_2d):
        # [128, 8c, 512] -> [128, 4hp, 8c, 128] (contiguous per-head-pair DMA)
        return np.ascontiguousarray(
            pack(w_2d).reshape(128, 8, 4, 128).transpose(0, 2, 1, 3)
        )

    qt = [pack(Q_seq[b].T[:, :Qcols]) for b in range(B)]
    kt = [pack(K_seq[b].T[:, :Kcols]) for b in range(B)]
    vt = []
    for b in range(B):
        v = np.asarray(V_seq[b]).copy()
        v[int(V_len[b, 0]) :] = 0.0  # masked keys contribute exactly 0
        vt.append(pack(v.T[:, :Kcols]))
    wkq = [
        np.ascontiguousarray(
            np.stack(
                [
                    pack_hp(WK[:, hh * NH : (hh + 1) * NH]),
                    pack_hp(WQ[:, hh * NH : (hh + 1) * NH]),
                ],
                axis=2,
            )
        )
        for hh in range(2)
    ]
    wv = [pack(WV[:, hh * NH : (hh + 1) * NH]) for hh in range(2)]
    in_maps = []
    for c in range(8):
        b, hh = c // 2, c % 2
        kones = (karange < int(V_len[b, 0])).astype(np.float32)
        in_maps.append(
            {
                "QT": qt[b],
                "KT": kt[b],
                "VT": vt[b],
                "Wkq": wkq[hh],
                "Wv": wv[hh],
                "kones": np.ascontiguousarray(
                    kones.reshape(KC, 128).T.astype(BF16_NP)
                ),
            }
        )
    return in_maps


def assemble(results, Q_len, plan):
    QC, _ = plan
    Qcols = 128 * QC
    out = np.zeros((B, S, H * D), np.float32)
    for c in range(8):
        b, hh = c // 2, c % 2
        # row layout: [4 hp, 65 p, 2 hx]; p=64 is the denominator row
        r = results[c]["OT"].reshape(4, 65, 2, Qcols)
        o = r[:, :64] / r[:, 64:65]  # [4, 64, 2, Qcols] normalized
        ql = int(Q_len[b, 0])
        n = min(ql, Qcols)
        # out col for head 2*hp+hx, dim d = (2*hp+hx)*64 + d
        out[b, :n, hh * NH : (hh + 1) * NH] = (
            o[:, :, :, :n].transpose(3, 0, 2, 1).reshape(n, NH)
        )
    return out


def kernel(Q_seq, K_seq, V_seq, WQ, WK, WV, Q_len, V_len):
    from concourse.bass_utils import run_bass_kernel_spmd

    Q_seq, K_seq, V_seq = (np.asarray(x, np.float32) for x in (Q_seq, K_seq, V_seq))
    WQ, WK, WV = (np.asarray(x, np.float32) for x in (WQ, WK, WV))
    Q_len, V_len = np.asarray(Q_len), np.asarray(V_len)
    plan = plan_from_lengths(Q_len, V_len)
    nc = get_nc(plan=plan)
    in_maps = make_in_maps(Q_seq, K_seq, V_seq, WQ, WK, WV, Q_len, V_len)
    r = run_bass_kernel_spmd(nc, in_maps, core_ids=list(range(8)))
    return assemble(r.results, Q_len, plan)
